# revision 1
# baseline (speedup 1.0000x reference)
"""Multi-head attention block (QKV proj + masked softmax + out proj + residual LN)
on 8 Trainium2 NeuronCores.

Sharding: 8 shards = (batch b, query-half); B=4, S=2048. Each core owns one
batch's full K/V and half its queries; no collectives, host concatenates.

Key compaction: masked keys contribute exactly 0 to the softmax numerator and
denominator, and key order inside the sums is irrelevant — so the host gathers
only the unmasked keys (<=1046 of 2048 here) and pads to S_KV=1280. Pad slots
get a -30000 exp bias -> exp underflows to exactly 0. Cuts score/exp/PV work
to 10/16 of full, mathematically exact.

Per-core strategy (all matmuls bf16 inputs, fp32 PSUM accumulation):
  - xT staged on host; projections contract d on partitions.
  - kT/qT per head-PAIR [128, S] (head h -> partitions (h%2)*64..);
    temperature and the k-bias are folded into the kT store.
  - scores transposed [k, q]: pad mask folded into exp bias, row sums via a
    ones-column in V. PSUM tiles 2 banks wide; one exp covers [128, 1024]
    (both q-tiles of a k-tile share the bias), halving ACT overhead.
  - PV contracts k on partitions; normalization = reciprocal_approx_fast of
    the sums row + gpsimd partition-broadcast + DVE multiply.
  - y = attn_out @ wo.T via K=128 head-pair contractions, then residual
    (+bo folded into x host-side, kept fp32) and LayerNorm (bn_stats/aggr).
  - Build-time specialization on the actual inputs: gamma==1/beta==0 and
    bv==0 drop their (otherwise dead) ops.
"""

import os
import numpy as np
import ml_dtypes

import concourse.bass as bass
import concourse.bacc as bacc
import concourse.tile as tile
import concourse.mybir as mybir
from concourse.bass_utils import run_bass_kernel_spmd

F32 = mybir.dt.float32
BF16 = mybir.dt.bfloat16
AF = mybir.ActivationFunctionType
ALU = mybir.AluOpType

B, S, D = 4, 2048, 512
H, HD = 8, 64
NCORES = 8
SQ = S // 2          # queries per core
NP = 4               # head pairs
NQT = SQ // 512      # 2 q-tiles of 512
NST = SQ // 128      # 8 output s-tiles

_CACHE = {}
LAST_RESULT = None


def _build(ln_trivial, bv_trivial, S_KV):
    NKT = S_KV // 128
    key = ("nc", ln_trivial, bv_trivial, S_KV)
    if key in _CACHE:
        return _CACHE[key]

    nc = bacc.Bacc("TRN2", target_bir_lowering=False, debug=False, num_devices=NCORES)

    xTk = nc.dram_tensor("xTk", [D, S_KV], BF16, kind="ExternalInput")
    xTq = nc.dram_tensor("xTq", [D, SQ], BF16, kind="ExternalInput")
    xq = nc.dram_tensor("xq", [SQ, D], F32, kind="ExternalInput")
    wqT = nc.dram_tensor("wqT", [D, D], BF16, kind="ExternalInput")
    wkT = nc.dram_tensor("wkT", [D, D], BF16, kind="ExternalInput")
    wvT = nc.dram_tensor("wvT", [D, D], BF16, kind="ExternalInput")
    woT = nc.dram_tensor("woT", [D, D], BF16, kind="ExternalInput")
    bqk = nc.dram_tensor("bqk", [128, 8], F32, kind="ExternalInput")
    if not bv_trivial:
        bv_row = nc.dram_tensor("bv_row", [1, D], BF16, kind="ExternalInput")
    maskb = nc.dram_tensor("maskb", [128, NKT], F32, kind="ExternalInput")
    temp_b = nc.dram_tensor("temp_b", [128, 1], F32, kind="ExternalInput")
    eye = nc.dram_tensor("eye", [128, 128], F32, kind="ExternalInput")
    if not ln_trivial:
        gamma = nc.dram_tensor("gamma", [1, D], F32, kind="ExternalInput")
        beta = nc.dram_tensor("beta", [1, D], F32, kind="ExternalInput")
    out = nc.dram_tensor("out", [SQ, D], F32, kind="ExternalOutput")

    def dram_bcast(t, p=128):
        a = t.ap()
        return bass.AP(tensor=a.tensor, offset=a.offset, ap=[[0, p]] + list(a.ap)[1:])

    with tile.TileContext(nc) as tc, nc.allow_low_precision(reason="bf16 matmuls"):
        with tc.tile_pool(name="consts", bufs=1) as consts, \
             tc.tile_pool(name="kqv", bufs=1) as kqv, \
             tc.tile_pool(name="proj", bufs=1) as proj, \
             tc.tile_pool(name="attn", bufs=3) as attn, \
             tc.tile_pool(name="psmm", bufs=3, space="PSUM") as psmm, \
             tc.tile_pool(name="pspv", bufs=2, space="PSUM") as pspv, \
             tc.tile_pool(name="small", bufs=2) as small:

            # ---- constants (small, fast DMAs first) ----
            bqk_t = consts.tile([128, 8], F32, tag="bqk")
            nc.sync.dma_start(out=bqk_t, in_=bqk[:, :])
            mb_t = consts.tile([128, NKT], F32, tag="mb")
            nc.sync.dma_start(out=mb_t, in_=maskb[:, :])
            tp_t = consts.tile([128, 1], F32, tag="tp")
            nc.sync.dma_start(out=tp_t, in_=temp_b[:, :])
            if not bv_trivial:
                bv_t = consts.tile([1, D], BF16, tag="bv")
                nc.sync.dma_start(out=bv_t, in_=bv_row[:, :])
            if not ln_trivial:
                g_t = consts.tile([128, D], F32, tag="g")
                nc.sync.dma_start(out=g_t, in_=dram_bcast(gamma))
                b_t = consts.tile([128, D], F32, tag="b")
                nc.sync.dma_start(out=b_t, in_=dram_bcast(beta))
            eye_t = consts.tile([128, 128], F32, tag="eye")
            nc.sync.dma_start(out=eye_t, in_=eye[:, :])
            eps_t = consts.tile([128, 1], F32, tag="eps")
            nc.vector.memset(eps_t, 1e-6)
            ones_f = consts.tile([128, 128], F32, tag="onesf")
            nc.vector.memset(ones_f, 1.0)
            ones_b = consts.tile([1, 128], BF16, tag="onesb")
            nc.vector.tensor_copy(out=ones_b, in_=ones_f[0:1, :])

            # ---- persistent activations ----
            kT = [kqv.tile([128, S_KV], BF16, tag=f"kT{p}", name=f"kT{p}")
                  for p in range(NP)]
            qT = [kqv.tile([128, SQ], BF16, tag=f"qT{p}", name=f"qT{p}")
                  for p in range(NP)]
            v_all = kqv.tile([128, H, NKT, HD + 1], BF16, tag="vall")
            outn = kqv.tile([128, NP, SQ], BF16, tag="outn")

            # ---- input staging: per-chunk DMAs so compute starts early ----
            wv_t = proj.tile([128, 4, D], BF16, tag="wv")
            xtk = proj.tile([128, 4, S_KV], BF16, tag="xtk")
            wk_t = proj.tile([128, 4, D], BF16, tag="wk")
            wq_t = proj.tile([128, 4, D], BF16, tag="wq")
            xtq = proj.tile([128, 4, SQ], BF16, tag="xtq")
            wo_t = consts.tile([128, 4, D], BF16, tag="wo")
            for c in range(4):
                nc.sync.dma_start(out=xtk[:, c, :], in_=xTk[c * 128:(c + 1) * 128, :])
            for c in range(4):
                nc.sync.dma_start(out=wv_t[:, c, :], in_=wvT[c * 128:(c + 1) * 128, :])
            for c in range(4):
                nc.sync.dma_start(out=wk_t[:, c, :], in_=wkT[c * 128:(c + 1) * 128, :])
            for c in range(4):
                nc.sync.dma_start(out=wq_t[:, c, :], in_=wqT[c * 128:(c + 1) * 128, :])
            for c in range(4):
                nc.sync.dma_start(out=xtq[:, c, :], in_=xTq[c * 128:(c + 1) * 128, :])
            for c in range(4):
                nc.sync.dma_start(out=wo_t[:, c, :], in_=woT[c * 128:(c + 1) * 128, :])

            # ---- PE warmup during input DMA wait: keeps the HAM clock-gate
            # open so the first real matmuls run at 2.4 GHz ----
            wu = consts.tile([128, 512], BF16, tag="wu")
            nc.vector.memset(wu, 0.0)
            wps = psmm.tile([128, 2, 512], F32, tag="mm", name="warm")
            for i in range(24):
                nc.tensor.matmul(wps[:, i % 2, :], wu[:, 0:128], wu,
                                 start=True, stop=True)

            # ---- V projection: all heads at once (+ ones-row bias matmul) ----
            nc.vector.tensor_copy(out=v_all[:, :, :, HD:HD + 1],
                                  in_=ones_f[:, 0:H * NKT])
            for t2 in range((NKT + 1) // 2):
                ts = [t for t in (2 * t2, 2 * t2 + 1) if t < NKT]
                ps = psmm.tile([128, 2, 512], F32, tag="mm")
                for j, t in enumerate(ts):
                    for c in range(4):
                        nc.tensor.matmul(
                            ps[:, j, :], xtk[:, c, t * 128:(t + 1) * 128],
                            wv_t[:, c, :], start=(c == 0),
                            stop=(c == 3 and bv_trivial))
                    if not bv_trivial:
                        nc.tensor.matmul(ps[:, j, :], ones_b[0:1, :], bv_t,
                                         start=False, stop=True)
                for h in range(H):
                    nc.vector.tensor_copy(
                        out=v_all[:, h, ts[0]:ts[0] + len(ts), 0:HD],
                        in_=ps[:, 0:len(ts), h * HD:(h + 1) * HD])

            def emit_kq(p):
                # kT store folds +bk and *temperature (exact when temp=2^-k)
                kv_groups = []
                off0 = 0
                while off0 < S_KV:
                    if S_KV - off0 >= 1024:
                        kv_groups.append((off0, (512, 512))); off0 += 1024
                    else:
                        kv_groups.append((off0, (S_KV - off0,))); off0 += S_KV - off0
                for g0, widths in kv_groups:
                    ps = psmm.tile([128, 2, 512], F32, tag="mm", name=f"psk{p}{g0}")
                    off = g0
                    for j, w in enumerate(widths):
                        for c in range(4):
                            nc.tensor.matmul(
                                ps[:, j, 0:w], wk_t[:, c, p * 128:(p + 1) * 128],
                                xtk[:, c, off:off + w],
                                start=(c == 0), stop=(c == 3))
                        off += w
                    tot = sum(widths)
                    src = ps if len(widths) == 2 else ps[:, 0, 0:tot]
                    nc.vector.tensor_scalar(
                        out=kT[p][:, g0:g0 + tot], in0=src,
                        scalar1=bqk_t[:, 4 + p:5 + p], scalar2=tp_t[:, 0:1],
                        op0=ALU.add, op1=ALU.mult)
                ps = psmm.tile([128, 2, 512], F32, tag="mm", name=f"psq{p}")
                for j in range(2):
                    for c in range(4):
                        nc.tensor.matmul(
                            ps[:, j, :], wq_t[:, c, p * 128:(p + 1) * 128],
                            xtq[:, c, j * 512:(j + 1) * 512],
                            start=(c == 0), stop=(c == 3))
                nc.vector.tensor_scalar_add(
                    out=qT[p][:, :], in0=ps, scalar1=bqk_t[:, p:p + 1])

            def emit_pv_chunk(prev, c):
                ph, pse, ppvs = prev
                for qt in range(NQT):
                    nc.tensor.matmul(
                        ppvs[qt], v_all[:, ph, c, :],
                        pse[:, c, qt * 512:(qt + 1) * 512],
                        start=(c == 0), stop=(c == NKT - 1))

            def emit_norm(prev):
                ph, pse, ppvs = prev
                php, phb = ph // 2, (ph % 2) * 64
                for qt in range(NQT):
                    sums = small.tile([1, 512], F32, tag="sums")
                    nc.vector.tensor_copy(out=sums, in_=ppvs[qt][HD:HD + 1, :])
                    rec = small.tile([1, 512], F32, tag="rec")
                    nc.vector.reciprocal_approx_fast(out=rec, in_=sums)
                    rec_b = small.tile([64, 512], F32, tag="recb")
                    nc.gpsimd.partition_broadcast(rec_b, rec)
                    nc.vector.tensor_mul(
                        outn[phb:phb + 64, php, qt * 512:(qt + 1) * 512],
                        ppvs[qt][0:HD, :], rec_b)

            # software pipeline: head h's score matmuls interleave with head
            # h-1's PV matmuls so the PE never drains while ACT runs exp.
            emit_kq(0)
            emit_kq(1)
            prev = None
            for p in range(NP):
                for h01 in range(2):
                    h = 2 * p + h01
                    hb = h01 * 64
                    se = attn.tile([128, NKT, SQ], BF16, tag="se", name=f"se{h}")
                    pvs = [pspv.tile([HD + 1, 512], F32, tag="pv",
                                     name=f"pv{h}_{qt}") for qt in range(NQT)]
                    for kt in range(NKT):
                        sps = psmm.tile([128, 2, 512], F32, tag="mm",
                                        name=f"sps{h}_{kt}")
                        for qt in range(NQT):
                            nc.tensor.matmul(
                                sps[:, qt, :],
                                kT[p][hb:hb + 64, kt * 128:(kt + 1) * 128],
                                qT[p][hb:hb + 64, qt * 512:(qt + 1) * 512],
                                start=True, stop=True)
                        if prev is not None:
                            emit_pv_chunk(prev, kt)
                        nc.scalar.activation(
                            out=se[:, kt, :], in_=sps, func=AF.Exp,
                            bias=mb_t[:, kt:kt + 1])
                    if prev is not None:
                        emit_norm(prev)
                    prev = (h, se, pvs)
                    if h01 == 1 and p + 2 < NP:
                        emit_kq(p + 2)
            for c in range(NKT):
                emit_pv_chunk(prev, c)
            emit_norm(prev)

            # ---- output projection + residual + LayerNorm ----
            xq_tiles = []
            for st in range(NST):
                xq_t = small.tile([128, D], F32, tag=f"xq{st}", name=f"xq{st}")
                nc.sync.dma_start(out=xq_t, in_=xq[st * 128:(st + 1) * 128, :])
                xq_tiles.append(xq_t)
            for st2 in range(NST // 2):
                yps = psmm.tile([128, 2, 512], F32, tag="mm", name=f"yps{st2}")
                for j in range(2):
                    st = 2 * st2 + j
                    for p in range(NP):
                        nc.tensor.matmul(
                            yps[:, j, :],
                            outn[:, p, st * 128:(st + 1) * 128],
                            wo_t[:, p, :],
                            start=(p == 0), stop=False)
                    nc.tensor.matmul(yps[:, j, :], eye_t, xq_tiles[st],
                                     start=False, stop=True)
                for j in range(2):
                    st = 2 * st2 + j
                    z = yps[:, j, :]
                    stats = small.tile([128, 6], F32, tag="stats")
                    nc.vector.bn_stats(out=stats, in_=z)
                    mv = small.tile([128, 2], F32, tag="mv")
                    nc.vector.bn_aggr(out=mv, in_=stats)
                    std = small.tile([128, 1], F32, tag="std")
                    nc.scalar.activation(out=std, in_=mv[:, 1:2], func=AF.Sqrt,
                                         bias=eps_t[:, 0:1])
                    rstd = small.tile([128, 1], F32, tag="rstd")
                    nc.vector.reciprocal(out=rstd, in_=std)
                    nb = small.tile([128, 1], F32, tag="nb")
                    nc.vector.tensor_scalar(
                        out=nb, in0=mv[:, 0:1], scalar1=rstd, scalar2=-1.0,
                        op0=ALU.mult, op1=ALU.mult)
                    zn = small.tile([128, D], F32, tag="zn")
                    nc.scalar.activation(out=zn, in_=z, func=AF.Identity,
                                         bias=nb[:, 0:1], scale=rstd[:, 0:1])
                    if ln_trivial:
                        zo = zn
                    else:
                        zg = small.tile([128, D], F32, tag="z")
                        nc.vector.tensor_mul(zg, zn, g_t)
                        zo = small.tile([128, D], F32, tag="zn2")
                        nc.vector.tensor_add(zo, zg, b_t)
                    nc.sync.dma_start(out=out[st * 128:(st + 1) * 128, :], in_=zo)

    nc.compile()
    _CACHE[key] = nc
    return nc


def _prep_in_maps(x, mask, wq, bq, wk, bk, wv, bv, wo, bo, ln_gamma, ln_beta,
                  temperature, ln_trivial, bv_trivial, S_KV):
    f32 = np.float32
    bf16 = ml_dtypes.bfloat16
    x = np.asarray(x, f32)
    mask = np.asarray(mask).astype(bool)
    wqT = np.ascontiguousarray(np.asarray(wq, f32).T).astype(bf16)
    wkT = np.ascontiguousarray(np.asarray(wk, f32).T).astype(bf16)
    wvT = np.ascontiguousarray(np.asarray(wv, f32).T).astype(bf16)
    woT = np.ascontiguousarray(np.asarray(wo, f32).T).astype(bf16)
    bq = np.asarray(bq, f32); bk = np.asarray(bk, f32)
    bv = np.asarray(bv, f32); bo = np.asarray(bo, f32)
    bqk = np.ascontiguousarray(
        np.concatenate([bq.reshape(4, 128).T, bk.reshape(4, 128).T], axis=1)
    ).astype(f32)
    temp_b = np.full((128, 1), np.asarray(temperature, f32).reshape(-1)[0], f32)

    in_maps = []
    for m in range(NCORES):
        b, half = m // 2, m % 2
        q0 = half * SQ
        xb = x[b]
        idx = np.where(~mask[b])[0]
        nkv = len(idx)
        assert nkv <= S_KV, f"unmasked keys {nkv} > S_KV={S_KV}"
        xk = np.zeros((S_KV, D), f32)
        xk[:nkv] = xb[idx]
        mbias = np.full(S_KV, -30000.0, f32)
        mbias[:nkv] = 0.0
        NKT = S_KV // 128
        im = {
            "eye": np.eye(128, dtype=f32),
            "xTk": np.ascontiguousarray(xk.T).astype(bf16),
            "xTq": np.ascontiguousarray(xb[q0:q0 + SQ].T).astype(bf16),
            "xq": np.ascontiguousarray(xb[q0:q0 + SQ] + bo[None, :]),
            "wqT": wqT, "wkT": wkT, "wvT": wvT, "woT": woT,
            "bqk": bqk,
            "maskb": np.ascontiguousarray(mbias.reshape(NKT, 128).T),
            "temp_b": temp_b,
        }
        if not bv_trivial:
            im["bv_row"] = bv.reshape(1, D).astype(bf16)
        if not ln_trivial:
            im["gamma"] = np.asarray(ln_gamma, f32).reshape(1, D)
            im["beta"] = np.asarray(ln_beta, f32).reshape(1, D)
        in_maps.append(im)
    return in_maps


def kernel(**inputs) -> np.ndarray:
    global LAST_RESULT
    ln_trivial = bool(np.all(np.asarray(inputs["ln_gamma"]) == 1.0)
                      and np.all(np.asarray(inputs["ln_beta"]) == 0.0))
    bv_trivial = bool(np.all(np.asarray(inputs["bv"]) == 0.0))
    maskarr = np.asarray(inputs["mask"]).astype(bool)
    max_unmasked = int((~maskarr).sum(axis=1).max())
    S_KV = max(256, -(-(max_unmasked + 64) // 128) * 128)
    nc = _build(ln_trivial, bv_trivial, S_KV)
    in_maps = _prep_in_maps(**inputs, ln_trivial=ln_trivial, bv_trivial=bv_trivial,
                            S_KV=S_KV)
    res = run_bass_kernel_spmd(nc, in_maps, core_ids=list(range(NCORES)),
                               trace=bool(os.environ.get("BASS_TRACE")))
    LAST_RESULT = res
    y = np.empty((B, S, D), np.float32)
    for m in range(NCORES):
        b, half = m // 2, m % 2
        y[b, half * SQ:(half + 1) * SQ] = res.results[m]["out"]
    return y



# revision 4
# speedup vs baseline: 1.2410x; 1.2410x over previous
"""Multi-head attention block (QKV proj + masked softmax + out proj + residual LN)
on 8 Trainium2 NeuronCores.

Sharding: 8 shards = (batch b, query-half); B=4, S=2048. Each core owns one
batch's full K/V and half its queries; no collectives, host concatenates.

Key compaction: masked keys contribute exactly 0 to the softmax numerator and
denominator, and key order inside the sums is irrelevant — so the host gathers
only the unmasked keys and pads to S_KV (multiple of 128). Pad slots get a
-30000 exp bias -> exp underflows to exactly 0. Mathematically exact.

Per-core strategy (all matmuls bf16 inputs, fp32 PSUM accumulation):
  - xT staged on host; projections contract d on partitions.
  - kT/qT per head-PAIR [128, S] (head h -> partitions (h%2)*64..);
    temperature folded into wq host-side (exact for power-of-2 temp).
  - scores transposed [k, q]: the two heads of a pair are issued as
    row-TILED matmuls (K=64 at array rows 0-63 / 64-127) so they run
    CONCURRENTLY on the PE; outputs land in adjacent PSUM banks and one
    exp [128, 1024] covers both heads (pad-mask bias is per-key, shared).
  - softmax row sums via a ones-column in V (M=65 PV matmuls).
  - PV contracts k on partitions; normalization = reciprocal_approx_fast
    of the sums row + gpsimd partition-broadcast + DVE multiply.
  - y = attn_out @ wo.T via K=128 head-pair contractions; residual added
    on DVE (z = psum + x + bo), then LayerNorm (bn_stats/aggr).
  - Build-time specialization on the actual inputs: gamma==1/beta==0,
    bv==0 and bq==bk==0 drop their (otherwise dead) ops.
"""

import os
import numpy as np
import ml_dtypes

import concourse.bass as bass
import concourse.bacc as bacc
import concourse.tile as tile
import concourse.mybir as mybir
from concourse.bass_utils import run_bass_kernel_spmd

F32 = mybir.dt.float32
BF16 = mybir.dt.bfloat16
AF = mybir.ActivationFunctionType
ALU = mybir.AluOpType

B, S, D = 4, 2048, 512
H, HD = 8, 64
NCORES = 8
SQ = S // 2          # queries per core
NP = 4               # head pairs
NST = SQ // 128      # 8 output s-tiles

_CACHE = {}
LAST_RESULT = None


def _build(ln_trivial, bv_trivial, qk_trivial, S_KV):
    NKT = S_KV // 128
    key = ("nc", ln_trivial, bv_trivial, qk_trivial, S_KV)
    if key in _CACHE:
        return _CACHE[key]

    nc = bacc.Bacc("TRN2", target_bir_lowering=False, debug=False, num_devices=NCORES)

    xTk = nc.dram_tensor("xTk", [D, S_KV], BF16, kind="ExternalInput")
    xTq = nc.dram_tensor("xTq", [D, SQ], BF16, kind="ExternalInput")
    xq = nc.dram_tensor("xq", [SQ, D], F32, kind="ExternalInput")
    wqT = nc.dram_tensor("wqT", [D, D], BF16, kind="ExternalInput")
    wkT = nc.dram_tensor("wkT", [D, D], BF16, kind="ExternalInput")
    wvT = nc.dram_tensor("wvT", [D, D], BF16, kind="ExternalInput")
    woT = nc.dram_tensor("woT", [D, D], BF16, kind="ExternalInput")
    if not qk_trivial:
        bqk = nc.dram_tensor("bqk", [128, 8], F32, kind="ExternalInput")
    if not bv_trivial:
        bv_row = nc.dram_tensor("bv_row", [1, D], BF16, kind="ExternalInput")
    maskb = nc.dram_tensor("maskb", [128, NKT], F32, kind="ExternalInput")
    if not ln_trivial:
        gamma = nc.dram_tensor("gamma", [1, D], F32, kind="ExternalInput")
        beta = nc.dram_tensor("beta", [1, D], F32, kind="ExternalInput")
    out = nc.dram_tensor("out", [SQ, D], F32, kind="ExternalOutput")

    def dram_bcast(t, p=128):
        a = t.ap()
        return bass.AP(tensor=a.tensor, offset=a.offset, ap=[[0, p]] + list(a.ap)[1:])

    with tile.TileContext(nc) as tc, nc.allow_low_precision(reason="bf16 matmuls"):
        with tc.tile_pool(name="consts", bufs=1) as consts, \
             tc.tile_pool(name="kqv", bufs=1) as kqv, \
             tc.tile_pool(name="proj", bufs=1) as proj, \
             tc.tile_pool(name="attn", bufs=3) as attn, \
             tc.tile_pool(name="psmm", bufs=2, space="PSUM") as psmm, \
             tc.tile_pool(name="pspv", bufs=4, space="PSUM") as pspv, \
             tc.tile_pool(name="small", bufs=2) as small:

            # ---- constants (small, fast DMAs first) ----
            if not qk_trivial:
                bqk_t = consts.tile([128, 8], F32, tag="bqk")
                nc.sync.dma_start(out=bqk_t, in_=bqk[:, :])
            mb_t = consts.tile([128, NKT], F32, tag="mb")
            nc.sync.dma_start(out=mb_t, in_=maskb[:, :])
            if not bv_trivial:
                bv_t = consts.tile([1, D], BF16, tag="bv")
                nc.sync.dma_start(out=bv_t, in_=bv_row[:, :])
            if not ln_trivial:
                g_t = consts.tile([128, D], F32, tag="g")
                nc.sync.dma_start(out=g_t, in_=dram_bcast(gamma))
                b_t = consts.tile([128, D], F32, tag="b")
                nc.sync.dma_start(out=b_t, in_=dram_bcast(beta))
            eps_t = consts.tile([128, 1], F32, tag="eps")
            nc.vector.memset(eps_t, 1e-6)
            ones_f = consts.tile([128, 128], F32, tag="onesf")
            nc.vector.memset(ones_f, 1.0)
            ones_b = consts.tile([1, 128], BF16, tag="onesb")
            nc.vector.tensor_copy(out=ones_b, in_=ones_f[0:1, :])

            # ---- persistent activations ----
            kT = [kqv.tile([128, S_KV], BF16, tag=f"kT{p}", name=f"kT{p}")
                  for p in range(NP)]
            qT = [kqv.tile([128, SQ], BF16, tag=f"qT{p}", name=f"qT{p}")
                  for p in range(NP)]
            v_all = kqv.tile([128, H, NKT, HD + 1], BF16, tag="vall")
            outn = kqv.tile([128, NP, SQ], BF16, tag="outn")

            # ---- input staging: per-chunk DMAs so compute starts early ----
            wv_t = proj.tile([128, 4, D], BF16, tag="wv")
            xtk = proj.tile([128, 4, S_KV], BF16, tag="xtk")
            wk_t = proj.tile([128, 4, D], BF16, tag="wk")
            wq_t = proj.tile([128, 4, D], BF16, tag="wq")
            xtq = proj.tile([128, 4, SQ], BF16, tag="xtq")
            wo_t = consts.tile([128, 4, D], BF16, tag="wo")
            for c in range(4):
                nc.sync.dma_start(out=xtk[:, c, :], in_=xTk[c * 128:(c + 1) * 128, :])
            for c in range(4):
                nc.sync.dma_start(out=wv_t[:, c, :], in_=wvT[c * 128:(c + 1) * 128, :])
            for c in range(4):
                nc.sync.dma_start(out=wk_t[:, c, :], in_=wkT[c * 128:(c + 1) * 128, :])
            for c in range(4):
                nc.sync.dma_start(out=wq_t[:, c, :], in_=wqT[c * 128:(c + 1) * 128, :])
            for c in range(4):
                nc.sync.dma_start(out=xtq[:, c, :], in_=xTq[c * 128:(c + 1) * 128, :])
            for c in range(4):
                nc.sync.dma_start(out=wo_t[:, c, :], in_=woT[c * 128:(c + 1) * 128, :])
            # residual input, needed only at the out-proj stage but DMA'd
            # early on otherwise-idle queues
            xq_tiles = []
            for st in range(NST):
                xq_t = small.tile([128, D], F32, tag=f"xq{st}", name=f"xq{st}")
                nc.sync.dma_start(out=xq_t, in_=xq[st * 128:(st + 1) * 128, :])
                xq_tiles.append(xq_t)

            # ---- PE warmup during input DMA wait: keeps the HAM clock-gate
            # open so the first real matmuls run at 2.4 GHz ----
            wu = consts.tile([128, 512], BF16, tag="wu")
            nc.vector.memset(wu, 0.0)
            wps = psmm.tile([128, 2, 512], F32, tag="mm", name="warm")
            for i in range(24):
                nc.tensor.matmul(wps[:, i % 2, :], wu[:, 0:128], wu,
                                 start=True, stop=True)

            # ---- V projection: all heads at once (+ ones-row bias matmul) ----
            nc.vector.tensor_copy(out=v_all[:, :, :, HD:HD + 1],
                                  in_=ones_f[:, 0:H * NKT])
            for t2 in range((NKT + 1) // 2):
                ts = [t for t in (2 * t2, 2 * t2 + 1) if t < NKT]
                ps = psmm.tile([128, 2, 512], F32, tag="mm")
                for j, t in enumerate(ts):
                    for c in range(4):
                        nc.tensor.matmul(
                            ps[:, j, :], xtk[:, c, t * 128:(t + 1) * 128],
                            wv_t[:, c, :], start=(c == 0),
                            stop=(c == 3 and bv_trivial))
                    if not bv_trivial:
                        nc.tensor.matmul(ps[:, j, :], ones_b[0:1, :], bv_t,
                                         start=False, stop=True)
                for h in range(H):
                    nc.vector.tensor_copy(
                        out=v_all[:, h, ts[0]:ts[0] + len(ts), 0:HD],
                        in_=ps[:, 0:len(ts), h * HD:(h + 1) * HD])

            def emit_kq(p):
                kv_groups = []
                off0 = 0
                while off0 < S_KV:
                    if S_KV - off0 >= 1024:
                        kv_groups.append((off0, (512, 512))); off0 += 1024
                    else:
                        kv_groups.append((off0, (S_KV - off0,))); off0 += S_KV - off0
                for g0, widths in kv_groups:
                    ps = psmm.tile([128, 2, 512], F32, tag="mm", name=f"psk{p}{g0}")
                    off = g0
                    for j, w in enumerate(widths):
                        for c in range(4):
                            nc.tensor.matmul(
                                ps[:, j, 0:w], wk_t[:, c, p * 128:(p + 1) * 128],
                                xtk[:, c, off:off + w],
                                start=(c == 0), stop=(c == 3))
                        off += w
                    tot = sum(widths)
                    src = ps if len(widths) == 2 else ps[:, 0, 0:tot]
                    if qk_trivial:
                        nc.vector.tensor_copy(out=kT[p][:, g0:g0 + tot], in_=src)
                    else:
                        nc.vector.tensor_scalar_add(
                            out=kT[p][:, g0:g0 + tot], in0=src,
                            scalar1=bqk_t[:, 4 + p:5 + p])
                ps = psmm.tile([128, 2, 512], F32, tag="mm", name=f"psq{p}")
                for j in range(2):
                    for c in range(4):
                        nc.tensor.matmul(
                            ps[:, j, :], wq_t[:, c, p * 128:(p + 1) * 128],
                            xtq[:, c, j * 512:(j + 1) * 512],
                            start=(c == 0), stop=(c == 3))
                if qk_trivial:
                    nc.vector.tensor_copy(out=qT[p][:, :], in_=ps)
                else:
                    nc.vector.tensor_scalar_add(
                        out=qT[p][:, :], in0=ps, scalar1=bqk_t[:, p:p + 1])

            def emit_pv_chunk(prev, kt):
                pp, pqt, pse, ppvs = prev
                for h01 in range(2):
                    nc.tensor.matmul(
                        ppvs[h01], v_all[:, 2 * pp + h01, kt, :],
                        pse[:, kt, h01, :],
                        start=(kt == 0), stop=(kt == NKT - 1))

            def emit_norm(prev):
                pp, pqt, pse, ppvs = prev
                for h01 in range(2):
                    hb = h01 * 64
                    sums = small.tile([1, 512], F32, tag="sums")
                    nc.vector.tensor_copy(out=sums, in_=ppvs[h01][HD:HD + 1, :])
                    rec = small.tile([1, 512], F32, tag="rec")
                    nc.vector.reciprocal_approx_fast(out=rec, in_=sums)
                    rec_b = small.tile([64, 512], F32, tag="recb")
                    nc.gpsimd.partition_broadcast(rec_b, rec)
                    nc.vector.tensor_mul(
                        outn[hb:hb + 64, pp, pqt * 512:(pqt + 1) * 512],
                        ppvs[h01][0:HD, :], rec_b)

            # software pipeline over 8 (pair, query-half) phases: the two
            # heads of a pair issue row-TILED score matmuls (concurrent on
            # the PE); phase i's PV matmuls interleave with phase i+1's
            # score matmuls so the PE never drains while ACT runs exp.
            emit_kq(0)
            emit_kq(1)
            prev = None
            for p in range(NP):
                for qt in range(2):
                    se_q = attn.tile([128, NKT, 2, 512], BF16, tag="se",
                                     name=f"se{p}_{qt}")
                    pvs = [pspv.tile([HD + 1, 512], F32, tag="pv",
                                     name=f"pv{p}{qt}_{h01}") for h01 in range(2)]
                    for kt in range(NKT):
                        sps = psmm.tile([128, 2, 512], F32, tag="mm",
                                        name=f"sps{p}{qt}_{kt}")
                        for h01 in range(2):
                            hb = h01 * 64
                            nc.tensor.matmul(
                                sps[:, h01, :],
                                kT[p][hb:hb + 64, kt * 128:(kt + 1) * 128],
                                qT[p][hb:hb + 64, qt * 512:(qt + 1) * 512],
                                start=True, stop=True)
                        if prev is not None:
                            emit_pv_chunk(prev, kt)
                        nc.scalar.activation(
                            out=se_q[:, kt, :, :], in_=sps, func=AF.Exp,
                            bias=mb_t[:, kt:kt + 1])
                    if prev is not None:
                        emit_norm(prev)
                    prev = (p, qt, se_q, pvs)
                    if qt == 1 and p + 2 < NP:
                        emit_kq(p + 2)
            for kt in range(NKT):
                emit_pv_chunk(prev, kt)
            emit_norm(prev)

            # ---- output projection + residual + LayerNorm ----
            for st2 in range(NST // 2):
                yps = psmm.tile([128, 2, 512], F32, tag="mm", name=f"yps{st2}")
                for j in range(2):
                    st = 2 * st2 + j
                    for p in range(NP):
                        nc.tensor.matmul(
                            yps[:, j, :],
                            outn[:, p, st * 128:(st + 1) * 128],
                            wo_t[:, p, :],
                            start=(p == 0), stop=(p == NP - 1))
                for j in range(2):
                    st = 2 * st2 + j
                    z = small.tile([128, D], F32, tag="z")
                    nc.vector.tensor_add(z, yps[:, j, :], xq_tiles[st])
                    stats = small.tile([128, 6], F32, tag="stats")
                    nc.vector.bn_stats(out=stats, in_=z)
                    mv = small.tile([128, 2], F32, tag="mv")
                    nc.vector.bn_aggr(out=mv, in_=stats)
                    std = small.tile([128, 1], F32, tag="std")
                    nc.scalar.activation(out=std, in_=mv[:, 1:2], func=AF.Sqrt,
                                         bias=eps_t[:, 0:1])
                    rstd = small.tile([128, 1], F32, tag="rstd")
                    nc.vector.reciprocal(out=rstd, in_=std)
                    nb = small.tile([128, 1], F32, tag="nb")
                    nc.vector.tensor_scalar(
                        out=nb, in0=mv[:, 0:1], scalar1=rstd, scalar2=-1.0,
                        op0=ALU.mult, op1=ALU.mult)
                    zn = small.tile([128, D], F32, tag="zn")
                    nc.scalar.activation(out=zn, in_=z, func=AF.Identity,
                                         bias=nb[:, 0:1], scale=rstd[:, 0:1])
                    if ln_trivial:
                        zo = zn
                    else:
                        zg = small.tile([128, D], F32, tag="zg")
                        nc.vector.tensor_mul(zg, zn, g_t)
                        zo = small.tile([128, D], F32, tag="zn2")
                        nc.vector.tensor_add(zo, zg, b_t)
                    nc.sync.dma_start(out=out[st * 128:(st + 1) * 128, :], in_=zo)

    nc.compile()
    _CACHE[key] = nc
    return nc


def _prep_in_maps(x, mask, wq, bq, wk, bk, wv, bv, wo, bo, ln_gamma, ln_beta,
                  temperature, ln_trivial, bv_trivial, qk_trivial, S_KV):
    f32 = np.float32
    bf16 = ml_dtypes.bfloat16
    x = np.asarray(x, f32)
    mask = np.asarray(mask).astype(bool)
    temp = np.asarray(temperature, f32).reshape(-1)[0]
    # fold temperature into wq (exact when temp is a power of two; the
    # commuted rounding error is ~2^-9 relative otherwise — negligible)
    wqT = np.ascontiguousarray(np.asarray(wq, f32).T * temp).astype(bf16)
    wkT = np.ascontiguousarray(np.asarray(wk, f32).T).astype(bf16)
    wvT = np.ascontiguousarray(np.asarray(wv, f32).T).astype(bf16)
    woT = np.ascontiguousarray(np.asarray(wo, f32).T).astype(bf16)
    bq = np.asarray(bq, f32) * temp
    bk = np.asarray(bk, f32)
    bv = np.asarray(bv, f32); bo = np.asarray(bo, f32)
    bqk = np.ascontiguousarray(
        np.concatenate([bq.reshape(4, 128).T, bk.reshape(4, 128).T], axis=1)
    ).astype(f32)

    in_maps = []
    for m in range(NCORES):
        b, half = m // 2, m % 2
        q0 = half * SQ
        xb = x[b]
        idx = np.where(~mask[b])[0]
        nkv = len(idx)
        assert nkv <= S_KV, f"unmasked keys {nkv} > S_KV={S_KV}"
        xk = np.zeros((S_KV, D), f32)
        xk[:nkv] = xb[idx]
        mbias = np.full(S_KV, -30000.0, f32)
        mbias[:nkv] = 0.0
        NKT = S_KV // 128
        im = {
            "xTk": np.ascontiguousarray(xk.T).astype(bf16),
            "xTq": np.ascontiguousarray(xb[q0:q0 + SQ].T).astype(bf16),
            "xq": np.ascontiguousarray(xb[q0:q0 + SQ] + bo[None, :]),
            "wqT": wqT, "wkT": wkT, "wvT": wvT, "woT": woT,
            "maskb": np.ascontiguousarray(mbias.reshape(NKT, 128).T),
        }
        if not qk_trivial:
            im["bqk"] = bqk
        if not bv_trivial:
            im["bv_row"] = bv.reshape(1, D).astype(bf16)
        if not ln_trivial:
            im["gamma"] = np.asarray(ln_gamma, f32).reshape(1, D)
            im["beta"] = np.asarray(ln_beta, f32).reshape(1, D)
        in_maps.append(im)
    return in_maps


def kernel(**inputs) -> np.ndarray:
    global LAST_RESULT
    ln_trivial = bool(np.all(np.asarray(inputs["ln_gamma"]) == 1.0)
                      and np.all(np.asarray(inputs["ln_beta"]) == 0.0))
    bv_trivial = bool(np.all(np.asarray(inputs["bv"]) == 0.0))
    qk_trivial = bool(np.all(np.asarray(inputs["bq"]) == 0.0)
                      and np.all(np.asarray(inputs["bk"]) == 0.0))
    maskarr = np.asarray(inputs["mask"]).astype(bool)
    max_unmasked = int((~maskarr).sum(axis=1).max())
    S_KV = max(256, -(-max_unmasked // 128) * 128)
    nc = _build(ln_trivial, bv_trivial, qk_trivial, S_KV)
    in_maps = _prep_in_maps(**inputs, ln_trivial=ln_trivial,
                            bv_trivial=bv_trivial, qk_trivial=qk_trivial,
                            S_KV=S_KV)
    res = run_bass_kernel_spmd(nc, in_maps, core_ids=list(range(NCORES)),
                               trace=bool(os.environ.get("BASS_TRACE")))
    LAST_RESULT = res
    y = np.empty((B, S, D), np.float32)
    for m in range(NCORES):
        b, half = m // 2, m % 2
        y[b, half * SQ:(half + 1) * SQ] = res.results[m]["out"]
    return y


# revision 7
# speedup vs baseline: 1.2499x; 1.0071x over previous
"""Multi-head attention block (QKV proj + masked softmax + out proj + residual LN)
on 8 Trainium2 NeuronCores.

Sharding: 8 shards = (batch b, query-half); B=4, S=2048. Each core owns one
batch's full K/V and half its queries; no collectives, host concatenates.

Key compaction: masked keys contribute exactly 0 to the softmax numerator and
denominator, and key order inside the sums is irrelevant — so the host gathers
only the unmasked keys and pads to S_KV (multiple of 128). Pad slots get a
-30000 exp bias -> exp underflows to exactly 0. Mathematically exact.

Per-core strategy (all matmuls bf16 inputs, fp32 PSUM accumulation):
  - xT staged on host; projections contract d on partitions.
  - kT/qT per head-PAIR [128, S] (head h -> partitions (h%2)*64..);
    temperature folded into wq host-side (exact for power-of-2 temp).
  - scores transposed [k, q]: the two heads of a pair are issued as
    row-TILED matmuls (K=64 at array rows 0-63 / 64-127) so they run
    CONCURRENTLY on the PE; outputs land in adjacent PSUM banks and one
    exp [128, 1024] covers both heads (pad-mask bias is per-key, shared).
  - softmax row sums via a ones-column in V (M=65 PV matmuls).
  - PV contracts k on partitions; normalization = reciprocal_approx_fast
    of the sums row + gpsimd partition-broadcast + DVE multiply.
  - y = attn_out @ wo.T via K=128 head-pair contractions; residual added
    on DVE (z = psum + x + bo), then LayerNorm (bn_stats/aggr).
  - Build-time specialization on the actual inputs: gamma==1/beta==0,
    bv==0 and bq==bk==0 drop their (otherwise dead) ops.
"""

import os
import numpy as np
import ml_dtypes

import concourse.bass as bass
import concourse.bacc as bacc
import concourse.tile as tile
import concourse.mybir as mybir
from concourse.bass_utils import run_bass_kernel_spmd

F32 = mybir.dt.float32
BF16 = mybir.dt.bfloat16
AF = mybir.ActivationFunctionType
ALU = mybir.AluOpType

B, S, D = 4, 2048, 512
H, HD = 8, 64
NCORES = 8
SQ = S // 2          # queries per core
NP = 4               # head pairs
NST = SQ // 128      # 8 output s-tiles

_CACHE = {}
LAST_RESULT = None


def _build(ln_trivial, bv_trivial, qk_trivial, S_KV):
    NKT = S_KV // 128
    key = ("nc", ln_trivial, bv_trivial, qk_trivial, S_KV)
    if key in _CACHE:
        return _CACHE[key]

    nc = bacc.Bacc("TRN2", target_bir_lowering=False, debug=False, num_devices=NCORES)

    xTk = nc.dram_tensor("xTk", [D, S_KV], BF16, kind="ExternalInput")
    xTq = nc.dram_tensor("xTq", [D, SQ], BF16, kind="ExternalInput")
    xq = nc.dram_tensor("xq", [SQ, D], F32, kind="ExternalInput")
    wqT = nc.dram_tensor("wqT", [D, D], BF16, kind="ExternalInput")
    wkT = nc.dram_tensor("wkT", [D, D], BF16, kind="ExternalInput")
    wvT = nc.dram_tensor("wvT", [D, D], BF16, kind="ExternalInput")
    woT = nc.dram_tensor("woT", [D, D], BF16, kind="ExternalInput")
    if not qk_trivial:
        bqk = nc.dram_tensor("bqk", [128, 8], F32, kind="ExternalInput")
    if not bv_trivial:
        bv_row = nc.dram_tensor("bv_row", [1, D], BF16, kind="ExternalInput")
    maskb = nc.dram_tensor("maskb", [128, NKT], F32, kind="ExternalInput")
    if not ln_trivial:
        gamma = nc.dram_tensor("gamma", [1, D], F32, kind="ExternalInput")
        beta = nc.dram_tensor("beta", [1, D], F32, kind="ExternalInput")
    out = nc.dram_tensor("out", [SQ, D], F32, kind="ExternalOutput")

    def dram_bcast(t, p=128):
        a = t.ap()
        return bass.AP(tensor=a.tensor, offset=a.offset, ap=[[0, p]] + list(a.ap)[1:])

    with tile.TileContext(nc) as tc, nc.allow_low_precision(reason="bf16 matmuls"):
        with tc.tile_pool(name="consts", bufs=1) as consts, \
             tc.tile_pool(name="kqv", bufs=1) as kqv, \
             tc.tile_pool(name="proj", bufs=1) as proj, \
             tc.tile_pool(name="attn", bufs=3) as attn, \
             tc.tile_pool(name="psmm", bufs=2, space="PSUM") as psmm, \
             tc.tile_pool(name="pspv", bufs=4, space="PSUM") as pspv, \
             tc.tile_pool(name="small", bufs=2) as small:

            # ---- constants (small, fast DMAs first) ----
            if not qk_trivial:
                bqk_t = consts.tile([128, 8], F32, tag="bqk")
                nc.sync.dma_start(out=bqk_t, in_=bqk[:, :])
            mb_t = consts.tile([128, NKT], F32, tag="mb")
            nc.sync.dma_start(out=mb_t, in_=maskb[:, :])
            if not bv_trivial:
                bv_t = consts.tile([1, D], BF16, tag="bv")
                nc.sync.dma_start(out=bv_t, in_=bv_row[:, :])
            if not ln_trivial:
                g_t = consts.tile([128, D], F32, tag="g")
                nc.sync.dma_start(out=g_t, in_=dram_bcast(gamma))
                b_t = consts.tile([128, D], F32, tag="b")
                nc.sync.dma_start(out=b_t, in_=dram_bcast(beta))
            eps_t = consts.tile([128, 1], F32, tag="eps")
            nc.vector.memset(eps_t, 1e-6)
            ones_f = consts.tile([128, 128], F32, tag="onesf")
            nc.vector.memset(ones_f, 1.0)
            ones_b = consts.tile([1, 128], BF16, tag="onesb")
            nc.vector.tensor_copy(out=ones_b, in_=ones_f[0:1, :])

            # ---- persistent activations ----
            kT = [kqv.tile([128, S_KV], BF16, tag=f"kT{p}", name=f"kT{p}")
                  for p in range(NP)]
            qT = [kqv.tile([128, SQ], BF16, tag=f"qT{p}", name=f"qT{p}")
                  for p in range(NP)]
            v_all = kqv.tile([128, H, NKT, HD + 1], BF16, tag="vall")
            outn = kqv.tile([128, NP, SQ], BF16, tag="outn")

            # ---- input staging: per-chunk DMAs ordered by first use so the
            # kq(0) -> scores -> exp pipeline starts as early as possible ----
            wv_t = proj.tile([128, 4, D], BF16, tag="wv")
            xtk = proj.tile([128, 4, S_KV], BF16, tag="xtk")
            wk_t = proj.tile([128, 4, D], BF16, tag="wk")
            wq_t = proj.tile([128, 4, D], BF16, tag="wq")
            xtq = proj.tile([128, 4, SQ], BF16, tag="xtq")
            wo_t = consts.tile([128, 4, D], BF16, tag="wo")
            for c in range(4):
                nc.sync.dma_start(out=xtk[:, c, :], in_=xTk[c * 128:(c + 1) * 128, :])
            for c in range(4):
                nc.sync.dma_start(out=wk_t[:, c, :], in_=wkT[c * 128:(c + 1) * 128, :])
            for c in range(4):
                nc.sync.dma_start(out=wq_t[:, c, :], in_=wqT[c * 128:(c + 1) * 128, :])
            for c in range(4):
                nc.sync.dma_start(out=xtq[:, c, :], in_=xTq[c * 128:(c + 1) * 128, :])
            for c in range(4):
                nc.sync.dma_start(out=wv_t[:, c, :], in_=wvT[c * 128:(c + 1) * 128, :])
            for c in range(4):
                nc.sync.dma_start(out=wo_t[:, c, :], in_=woT[c * 128:(c + 1) * 128, :])
            # residual input, needed only at the out-proj stage but DMA'd
            # early on otherwise-idle queues
            xq_tiles = []
            for st in range(NST):
                xq_t = small.tile([128, D], F32, tag=f"xq{st}", name=f"xq{st}")
                nc.sync.dma_start(out=xq_t, in_=xq[st * 128:(st + 1) * 128, :])
                xq_tiles.append(xq_t)

            # preload the exp ACT table during the DMA wait (walrus places
            # the table load before the first activation in program order)
            dmy = consts.tile([1, 8], F32, tag="dmy")
            nc.scalar.activation(out=dmy, in_=ones_f[0:1, 0:8], func=AF.Exp)

            # ---- PE warmup during input DMA wait: keeps the HAM clock-gate
            # open so the first real matmuls run at 2.4 GHz ----
            wu = consts.tile([128, 512], BF16, tag="wu")
            nc.vector.memset(wu, 0.0)
            wps = psmm.tile([128, 2, 512], F32, tag="mm", name="warm")
            for i in range(24):
                nc.tensor.matmul(wps[:, i % 2, :], wu[:, 0:128], wu,
                                 start=True, stop=True)

            nc.vector.tensor_copy(out=v_all[:, :, :, HD:HD + 1],
                                  in_=ones_f[:, 0:H * NKT])

            # ---- V projection piece (two key-tiles, all heads at once,
            # + ones-row bias matmul when bv is nontrivial) ----
            def v_part(t2):
                def f():
                    ts = [t for t in (2 * t2, 2 * t2 + 1) if t < NKT]
                    ps = psmm.tile([128, 2, 512], F32, tag="mm", name=f"psv{t2}")
                    for j, t in enumerate(ts):
                        for c in range(4):
                            nc.tensor.matmul(
                                ps[:, j, :], xtk[:, c, t * 128:(t + 1) * 128],
                                wv_t[:, c, :], start=(c == 0),
                                stop=(c == 3 and bv_trivial))
                        if not bv_trivial:
                            nc.tensor.matmul(ps[:, j, :], ones_b[0:1, :], bv_t,
                                             start=False, stop=True)
                    for h in range(H):
                        nc.vector.tensor_copy(
                            out=v_all[:, h, ts[0]:ts[0] + len(ts), 0:HD],
                            in_=ps[:, 0:len(ts), h * HD:(h + 1) * HD])
                return f

            def kq_k_part(p, g0, widths):
                def f():
                    ps = psmm.tile([128, 2, 512], F32, tag="mm", name=f"psk{p}{g0}")
                    off = g0
                    for j, w in enumerate(widths):
                        for c in range(4):
                            nc.tensor.matmul(
                                ps[:, j, 0:w], wk_t[:, c, p * 128:(p + 1) * 128],
                                xtk[:, c, off:off + w],
                                start=(c == 0), stop=(c == 3))
                        off += w
                    tot = sum(widths)
                    src = ps if len(widths) == 2 else ps[:, 0, 0:tot]
                    if qk_trivial:
                        nc.vector.tensor_copy(out=kT[p][:, g0:g0 + tot], in_=src)
                    else:
                        nc.vector.tensor_scalar_add(
                            out=kT[p][:, g0:g0 + tot], in0=src,
                            scalar1=bqk_t[:, 4 + p:5 + p])
                return f

            def kq_q_part(p):
                def f():
                    ps = psmm.tile([128, 2, 512], F32, tag="mm", name=f"psq{p}")
                    for j in range(2):
                        for c in range(4):
                            nc.tensor.matmul(
                                ps[:, j, :], wq_t[:, c, p * 128:(p + 1) * 128],
                                xtq[:, c, j * 512:(j + 1) * 512],
                                start=(c == 0), stop=(c == 3))
                    if qk_trivial:
                        nc.vector.tensor_copy(out=qT[p][:, :], in_=ps)
                    else:
                        nc.vector.tensor_scalar_add(
                            out=qT[p][:, :], in0=ps, scalar1=bqk_t[:, p:p + 1])
                return f

            def kq_parts(p):
                parts = []
                off0 = 0
                while off0 < S_KV:
                    if S_KV - off0 >= 1024:
                        parts.append(kq_k_part(p, off0, (512, 512)))
                        off0 += 1024
                    else:
                        parts.append(kq_k_part(p, off0, (S_KV - off0,)))
                        off0 += S_KV - off0
                parts.append(kq_q_part(p))
                return parts

            def emit_pv_chunk(prev, kt):
                pp, pqt, pse, ppvs = prev
                for h01 in range(2):
                    nc.tensor.matmul(
                        ppvs[h01], v_all[:, 2 * pp + h01, kt, :],
                        pse[:, kt, h01, :],
                        start=(kt == 0), stop=(kt == NKT - 1))

            def emit_norm(prev):
                pp, pqt, pse, ppvs = prev
                for h01 in range(2):
                    hb = h01 * 64
                    sums = small.tile([1, 512], F32, tag="sums")
                    nc.vector.tensor_copy(out=sums, in_=ppvs[h01][HD:HD + 1, :])
                    rec = small.tile([1, 512], F32, tag="rec")
                    nc.vector.reciprocal_approx_fast(out=rec, in_=sums)
                    rec_b = small.tile([64, 512], F32, tag="recb")
                    nc.gpsimd.partition_broadcast(rec_b, rec)
                    nc.vector.tensor_mul(
                        outn[hb:hb + 64, pp, pqt * 512:(pqt + 1) * 512],
                        ppvs[h01][0:HD, :], rec_b)

            def emit_outproj(st2):
                yps = psmm.tile([128, 2, 512], F32, tag="mm", name=f"yps{st2}")
                for j in range(2):
                    st = 2 * st2 + j
                    for p in range(NP):
                        nc.tensor.matmul(
                            yps[:, j, :],
                            outn[:, p, st * 128:(st + 1) * 128],
                            wo_t[:, p, :],
                            start=(p == 0), stop=(p == NP - 1))
                for j in range(2):
                    st = 2 * st2 + j
                    z = small.tile([128, D], F32, tag="z")
                    nc.vector.tensor_add(z, yps[:, j, :], xq_tiles[st])
                    stats = small.tile([128, 6], F32, tag="stats")
                    nc.vector.bn_stats(out=stats, in_=z)
                    mv = small.tile([128, 2], F32, tag="mv")
                    nc.vector.bn_aggr(out=mv, in_=stats)
                    std = small.tile([128, 1], F32, tag="std")
                    nc.scalar.activation(out=std, in_=mv[:, 1:2], func=AF.Sqrt,
                                         bias=eps_t[:, 0:1])
                    rstd = small.tile([128, 1], F32, tag="rstd")
                    nc.vector.reciprocal(out=rstd, in_=std)
                    nb = small.tile([128, 1], F32, tag="nb")
                    nc.vector.tensor_scalar(
                        out=nb, in0=mv[:, 0:1], scalar1=rstd, scalar2=-1.0,
                        op0=ALU.mult, op1=ALU.mult)
                    zn = small.tile([128, D], F32, tag="zn")
                    nc.scalar.activation(out=zn, in_=z, func=AF.Identity,
                                         bias=nb[:, 0:1], scale=rstd[:, 0:1])
                    if ln_trivial:
                        zo = zn
                    else:
                        zg = small.tile([128, D], F32, tag="zg")
                        nc.vector.tensor_mul(zg, zn, g_t)
                        zo = small.tile([128, D], F32, tag="zn2")
                        nc.vector.tensor_add(zo, zg, b_t)
                    nc.sync.dma_start(out=out[st * 128:(st + 1) * 128, :], in_=zo)

            # software pipeline over 8 (pair, query-half) phases: the two
            # heads of a pair issue row-TILED score matmuls (concurrent on
            # the PE); phase i's PV matmuls interleave with phase i+1's
            # score matmuls so the PE never drains while ACT runs exp.
            # kt slots are processed in pairs (fewer 64<->128 array-mode
            # flips); one deferred V-proj/kq piece slips in after each kt
            # pair as PE filler under the exp stream.
            kt_groups = [tuple(t for t in (2 * g, 2 * g + 1) if t < NKT)
                         for g in range((NKT + 1) // 2)]
            ngrp = len(kt_groups)
            fillers = kq_parts(1) + [v_part(t2) for t2 in range(ngrp)] \
                + kq_parts(2) + kq_parts(3)
            if ngrp < 4:  # tiny-S_KV fallback: V must fully precede phase 1
                for fl in fillers:
                    fl()
                fillers = []
            for fl in kq_parts(0):
                fl()
            fi = 0
            prev = None
            for phase_idx in range(2 * NP):
                p, qt = phase_idx // 2, phase_idx % 2
                last = phase_idx == 2 * NP - 1
                se_q = attn.tile([128, NKT, 2, 512], BF16, tag="se",
                                 name=f"se{p}_{qt}")
                pvs = [pspv.tile([HD + 1, 512], F32, tag="pv",
                                 name=f"pv{p}{qt}_{h01}") for h01 in range(2)]
                if last:
                    # front-load prev's PV into the first 3 groups so its
                    # norm (and the first half of the out projection) can
                    # overlap the remaining exp stream
                    chunks = list(range(NKT))
                    third = -(-NKT // 3)
                    pv_plan = [tuple(chunks[i * third:(i + 1) * third])
                               for i in range(3)] + [()] * (ngrp - 3)
                else:
                    pv_plan = kt_groups
                prev_normed = False
                for gi, g in enumerate(kt_groups):
                    for kt in g:
                        sps = psmm.tile([128, 2, 512], F32, tag="mm",
                                        name=f"sps{p}{qt}_{kt}")
                        for h01 in range(2):
                            hb = h01 * 64
                            nc.tensor.matmul(
                                sps[:, h01, :],
                                kT[p][hb:hb + 64, kt * 128:(kt + 1) * 128],
                                qT[p][hb:hb + 64, qt * 512:(qt + 1) * 512],
                                start=True, stop=True)
                        nc.scalar.activation(
                            out=se_q[:, kt, :, :], in_=sps, func=AF.Exp,
                            bias=mb_t[:, kt:kt + 1])
                    if prev is not None:
                        for kt2 in pv_plan[gi]:
                            emit_pv_chunk(prev, kt2)
                    if last and prev is not None:
                        if gi == 2:
                            emit_norm(prev)
                            prev_normed = True
                        elif gi == 3:
                            emit_outproj(0)
                        elif gi == 4:
                            emit_outproj(1)
                    elif fi < len(fillers):
                        fillers[fi]()
                        fi += 1
                if prev is not None and not prev_normed:
                    emit_norm(prev)
                prev = (p, qt, se_q, pvs)
            for kt in range(NKT):
                emit_pv_chunk(prev, kt)
            emit_norm(prev)
            emit_outproj(2)
            emit_outproj(3)

    nc.compile()
    _CACHE[key] = nc
    return nc


def _prep_in_maps(x, mask, wq, bq, wk, bk, wv, bv, wo, bo, ln_gamma, ln_beta,
                  temperature, ln_trivial, bv_trivial, qk_trivial, S_KV):
    f32 = np.float32
    bf16 = ml_dtypes.bfloat16
    x = np.asarray(x, f32)
    mask = np.asarray(mask).astype(bool)
    temp = np.asarray(temperature, f32).reshape(-1)[0]
    # fold temperature into wq (exact when temp is a power of two; the
    # commuted rounding error is ~2^-9 relative otherwise — negligible)
    wqT = np.ascontiguousarray(np.asarray(wq, f32).T * temp).astype(bf16)
    wkT = np.ascontiguousarray(np.asarray(wk, f32).T).astype(bf16)
    wvT = np.ascontiguousarray(np.asarray(wv, f32).T).astype(bf16)
    woT = np.ascontiguousarray(np.asarray(wo, f32).T).astype(bf16)
    bq = np.asarray(bq, f32) * temp
    bk = np.asarray(bk, f32)
    bv = np.asarray(bv, f32); bo = np.asarray(bo, f32)
    bqk = np.ascontiguousarray(
        np.concatenate([bq.reshape(4, 128).T, bk.reshape(4, 128).T], axis=1)
    ).astype(f32)

    in_maps = []
    for m in range(NCORES):
        b, half = m // 2, m % 2
        q0 = half * SQ
        xb = x[b]
        idx = np.where(~mask[b])[0]
        nkv = len(idx)
        assert nkv <= S_KV, f"unmasked keys {nkv} > S_KV={S_KV}"
        xk = np.zeros((S_KV, D), f32)
        xk[:nkv] = xb[idx]
        mbias = np.full(S_KV, -30000.0, f32)
        mbias[:nkv] = 0.0
        NKT = S_KV // 128
        im = {
            "xTk": np.ascontiguousarray(xk.T).astype(bf16),
            "xTq": np.ascontiguousarray(xb[q0:q0 + SQ].T).astype(bf16),
            "xq": np.ascontiguousarray(xb[q0:q0 + SQ] + bo[None, :]),
            "wqT": wqT, "wkT": wkT, "wvT": wvT, "woT": woT,
            "maskb": np.ascontiguousarray(mbias.reshape(NKT, 128).T),
        }
        if not qk_trivial:
            im["bqk"] = bqk
        if not bv_trivial:
            im["bv_row"] = bv.reshape(1, D).astype(bf16)
        if not ln_trivial:
            im["gamma"] = np.asarray(ln_gamma, f32).reshape(1, D)
            im["beta"] = np.asarray(ln_beta, f32).reshape(1, D)
        in_maps.append(im)
    return in_maps


def kernel(**inputs) -> np.ndarray:
    global LAST_RESULT
    ln_trivial = bool(np.all(np.asarray(inputs["ln_gamma"]) == 1.0)
                      and np.all(np.asarray(inputs["ln_beta"]) == 0.0))
    bv_trivial = bool(np.all(np.asarray(inputs["bv"]) == 0.0))
    qk_trivial = bool(np.all(np.asarray(inputs["bq"]) == 0.0)
                      and np.all(np.asarray(inputs["bk"]) == 0.0))
    maskarr = np.asarray(inputs["mask"]).astype(bool)
    max_unmasked = int((~maskarr).sum(axis=1).max())
    S_KV = max(256, -(-max_unmasked // 128) * 128)
    nc = _build(ln_trivial, bv_trivial, qk_trivial, S_KV)
    in_maps = _prep_in_maps(**inputs, ln_trivial=ln_trivial,
                            bv_trivial=bv_trivial, qk_trivial=qk_trivial,
                            S_KV=S_KV)
    res = run_bass_kernel_spmd(nc, in_maps, core_ids=list(range(NCORES)),
                               trace=bool(os.environ.get("BASS_TRACE")))
    LAST_RESULT = res
    y = np.empty((B, S, D), np.float32)
    for m in range(NCORES):
        b, half = m // 2, m % 2
        y[b, half * SQ:(half + 1) * SQ] = res.results[m]["out"]
    return y
